# revision 10
# baseline (speedup 1.0000x reference)
"""Multi-head self-attention (B=2, S=2048, D=1024, H=16) on 8 TRN2 NeuronCores.

Sharding: data-parallel over batch (2) x tensor-parallel over head-groups (4).
Core c = b*4 + hg handles batch b, heads hg*4..hg*4+3 (4 heads, 256 features).

Per-core device program (SPMD, identical on all cores):
  - QKV projections for the core's 256 output features (column-parallel)
  - full S x S attention for its 4 heads (softmax without max-subtraction,
    denominators via an appended ones-column in the PV matmul)
  - partial output projection (row-parallel): out_partial^T [1024, 2048]
Host: shards/transposes inputs, sums the 4 partial outputs per batch
(the "all-reduce"), adds bo, and untransposes.

All matmuls run in bf16 on the PE; accumulation is fp32 in PSUM.

Schedule notes (from trace analysis):
  - DMA triggers cost ~0.6us each serialized on their queue; loads are split
    across the SP and ACT hardware-DGE queues with the kh critical path
    (wk, xk) first, and weight loads batched into one trigger each.
  - Output stores are batched per s-block (1 trigger instead of 8).
  - The softmax Exp on the ACT engine (1 elem/lane/cycle @1.2GHz) paces the
    attention phase; scores matmul pairs run concurrently via tile_position
    row-tiling, PV fills the gaps with a 3-chunk lag pipeline.
  - vh is padded to 128 columns (ones at col 64, zeros above) so the PV
    LDWEIGHTS takes the fast-weight-load path.
"""

import numpy as np

B, S, D = 2, 2048, 1024
H, DK = 16, 64
NCORES = 8
HG = 4          # head groups (tensor parallel)
HPG = 4         # heads per group
F = HPG * DK    # 256 local features per core
SCALE = 1.0 / np.sqrt(DK)

_compiled = {}


def _build():
    import concourse.bacc as bacc
    import concourse.tile as tile
    from concourse import mybir

    f32 = mybir.dt.float32
    bf16 = mybir.dt.bfloat16
    Exp = mybir.ActivationFunctionType.Exp
    mult = mybir.AluOpType.mult

    nc = bacc.Bacc("TRN2", target_bir_lowering=False, debug=False,
                   enable_asserts=True, num_devices=NCORES)

    xq = nc.dram_tensor("xq", (D, S), bf16, kind="ExternalInput")   # q[b].T
    xk = nc.dram_tensor("xk", (D, S), bf16, kind="ExternalInput")
    xv = nc.dram_tensor("xv", (D, S), bf16, kind="ExternalInput")
    wq = nc.dram_tensor("wq", (D, F), bf16, kind="ExternalInput")   # Wq[rows].T
    wk = nc.dram_tensor("wk", (D, F), bf16, kind="ExternalInput")
    wv = nc.dram_tensor("wv", (D, F), bf16, kind="ExternalInput")
    wo = nc.dram_tensor("wo", (F, D), bf16, kind="ExternalInput")   # Wo[:, cols].T
    bq = nc.dram_tensor("bq", (128, 2), f32, kind="ExternalInput")  # bias, f-tiled
    bk = nc.dram_tensor("bk", (128, 2), f32, kind="ExternalInput")
    bv = nc.dram_tensor("bv", (1, F), f32, kind="ExternalInput")
    out = nc.dram_tensor("out", (D, S), bf16, kind="ExternalOutput")  # partial^T

    NDT = D // 128   # 8 d-tiles
    NST = S // 128   # 16 s-tiles (j tiles)
    NSB = S // 512   # 4 s-blocks (i blocks)
    PVLAG = 3        # PV chunk-lag behind scores (chunks of 2 jt)

    with tile.TileContext(nc) as tc:
        import contextlib
        with contextlib.ExitStack() as ctx:
            consts = ctx.enter_context(tc.tile_pool(name="consts", bufs=1))
            big = ctx.enter_context(tc.tile_pool(name="big", bufs=24))
            atp = ctx.enter_context(tc.tile_pool(name="atp", bufs=9))
            acts = ctx.enter_context(tc.tile_pool(name="acts", bufs=1))
            ostage = ctx.enter_context(tc.tile_pool(name="ostage", bufs=2))
            small = ctx.enter_context(tc.tile_pool(name="small", bufs=1))
            ps = ctx.enter_context(tc.tile_pool(name="ps", bufs=1, space="PSUM"))

            # ---- constants (each weight loads with a single DMA trigger) ----
            wq_sb = consts.tile([128, NDT, F], bf16, tag="wq")
            wk_sb = consts.tile([128, NDT, F], bf16, tag="wk")
            wv_sb = consts.tile([128, NDT, F], bf16, tag="wv")
            wo_sb = consts.tile([128, 2, D], bf16, tag="wo")
            bq_sb = consts.tile([128, 2], f32, tag="bq")
            bk_sb = consts.tile([128, 2], f32, tag="bk")
            bv_sb = consts.tile([128, F], f32, tag="bv")

            # persistent activations
            # qh/kh: [f, s] transposed projections, per (ft, sb) tiles
            qh_t = [[acts.tile([128, 512], bf16, tag=f"qh{ft}{sb}", name=f"qh{ft}{sb}")
                     for sb in range(NSB)] for ft in range(2)]
            kh_t = [[acts.tile([128, 512], bf16, tag=f"kh{ft}{sb}", name=f"kh{ft}{sb}")
                     for sb in range(NSB)] for ft in range(2)]
            # vh: [s, h, c] padded to 128 cols: values at 0..63, ones at 64
            # (PV denominator trick), zeros at 65..127 so LDWEIGHTS uses the
            # 128-column fast path.
            vh_t = [acts.tile([128, HPG, 128], bf16, tag=f"vh{st}", name=f"vh{st}")
                    for st in range(NST)]
            for st in range(NST):
                nc.vector.memset(vh_t[st][:, :, DK:], 1.0)
            # y: normalized attention output, [f, s] per (ft, sb)
            y_t = [[acts.tile([128, 512], bf16, tag=f"y{ft}{sb}", name=f"y{ft}{sb}")
                    for sb in range(NSB)] for ft in range(2)]

            # ---- input DMAs: kh critical path (wk, xk) on the ACT hwdge
            # queue, everything else on SP, both issuing from t=0 ----
            def alloc_x(pfx):
                return [big.tile([128, S], bf16, tag="big", name=f"{pfx}{dt}")
                        for dt in range(NDT)]

            xkt = alloc_x("xk")
            xqt = alloc_x("xq")
            xvt = alloc_x("xv")

            def w_src(wdram):
                return wdram.ap().rearrange("(dt p) f -> p dt f", p=128)

            # keys first (QK needs all of them)
            nc.sync.dma_start(wk_sb[:], w_src(wk))
            for dt in range(NDT):
                nc.sync.dma_start(xkt[dt][:], xk.ap()[dt * 128:(dt + 1) * 128, :])
            nc.sync.dma_start(bk_sb[:], bk.ap()[:])
            # SP queue: queries (first half), then values, then the rest
            nc.sync.dma_start(wq_sb[:], w_src(wq))
            nc.sync.dma_start(bq_sb[:], bq.ap()[:])
            for dt in range(NDT):
                nc.sync.dma_start(xqt[dt][:, 0:S // 2],
                                  xq.ap()[dt * 128:(dt + 1) * 128, 0:S // 2])
            nc.sync.dma_start(wv_sb[:], w_src(wv))
            nc.sync.dma_start(bv_sb[:], bv.ap().to_broadcast((128, F)))
            for dt in range(NDT):
                nc.sync.dma_start(xvt[dt][:], xv.ap()[dt * 128:(dt + 1) * 128, :])
            for dt in range(NDT):
                nc.sync.dma_start(xqt[dt][:, S // 2:S],
                                  xq.ap()[dt * 128:(dt + 1) * 128, S // 2:S])
            nc.sync.dma_start(wo_sb[:], wo.ap().rearrange("(ft p) d -> p ft d", p=128))

            # ---- projection pass helpers (per (ft, sb) single-bank accum) ----
            def qk_pass(w_sb, b_sb, xts, dst, ft, pfx, gs=(0, 1)):
                for g in gs:
                    accs = [ps.tile([128, 512], f32, tag="w1", bufs=4,
                                    name=f"{pfx}{ft}{g}{j}") for j in range(2)]
                    for dt in range(NDT):
                        for j in range(2):
                            sb = 2 * g + j
                            nc.tensor.matmul(
                                accs[j][:],
                                w_sb[:, dt, ft * 128:(ft + 1) * 128],
                                xts[dt][:, sb * 512:(sb + 1) * 512],
                                start=(dt == 0), stop=(dt == NDT - 1),
                            )
                    for j in range(2):
                        nc.vector.tensor_scalar_add(dst[ft][2 * g + j][:], accs[j][:],
                                                    b_sb[:, ft:ft + 1])

            def v_pass(xvt):
                for g in range(NST // 2):
                    accs = [ps.tile([128, 512], f32, tag="w1", bufs=4,
                                    name=f"vps{g}{j}") for j in range(2)]
                    for dt in range(NDT):
                        for j in range(2):
                            st = 2 * g + j
                            nc.tensor.matmul(
                                accs[j][:, 0:F],
                                xvt[dt][:, st * 128:(st + 1) * 128],
                                wv_sb[:, dt, :],
                                start=(dt == 0), stop=(dt == NDT - 1),
                            )
                    for j in range(2):
                        st = 2 * g + j
                        nc.vector.tensor_tensor(
                            vh_t[st][:, :, 0:DK],
                            accs[j][:, 0:F].rearrange("p (h c) -> p h c", h=HPG),
                            bv_sb[:].rearrange("p (h c) -> p h c", h=HPG),
                            mybir.AluOpType.add,
                        )

            # ---- attention pipeline: PV lags scores by PVLAG chunks so the
            # ACT-paced scores stream always has PE filler work ----
            def scores_chunk(pr, ib, jc):
                ft = pr
                at = atp.tile([128, 4, 512], bf16, tag="at", name=f"at{pr}{ib}{jc}")
                for jj in range(2):
                    jt = jc * 2 + jj
                    sc = ps.tile([128, 2, 512], f32, tag="w2", bufs=2, name="sc")
                    for hh in range(2):
                        base = hh * 64
                        nc.tensor.matmul(
                            sc[:, hh, :],
                            kh_t[ft][jt // 4][base:base + 64,
                                              (jt % 4) * 128:(jt % 4 + 1) * 128],
                            qh_t[ft][ib][base:base + 64, :],
                            start=True, stop=True,
                            tile_position=(base, 0),
                        )
                    nc.scalar.activation(
                        at[:, jj * 2:jj * 2 + 2, :],
                        sc[:, :, :],
                        Exp, scale=float(SCALE),
                    )
                return at

            def pv_chunk(pr, pv_ps, at, jc):
                for hh in range(2):
                    h = 2 * pr + hh
                    for jj in range(2):
                        jt = 2 * jc + jj
                        nc.tensor.matmul(
                            pv_ps[hh][:],
                            vh_t[jt][:, h, :],
                            at[:, 2 * jj + hh, :],
                            start=(jt == 0), stop=(jt == NST - 1),
                        )

            def finish_ib(pr, ib, pv_ps, with_outproj=False):
                ft = pr
                for hh in range(2):
                    den = small.tile([1, 512], f32, tag="den")
                    nc.vector.tensor_copy(den[:], pv_ps[hh][DK:DK + 1, :])
                    rec = small.tile([1, 512], f32, tag="rec")
                    nc.vector.reciprocal_approx_fast(rec[:], den[:])
                    rb = small.tile([64, 512], f32, tag="rb")
                    nc.gpsimd.partition_broadcast(rb[:], rec[:])
                    nc.vector.tensor_tensor(
                        y_t[ft][ib][hh * 64:hh * 64 + 64, :],
                        pv_ps[hh][0:DK, :],
                        rb[:],
                        mult,
                    )
                if with_outproj:
                    outproj_sb(ib)

            # ---- output projection for one s-block (single batched store) ----
            def outproj_sb(sb):
                for eh in range(2):
                    o_sb = ostage.tile([128, NDT // 2, 512], bf16, tag="ost",
                                       name=f"os{sb}{eh}")
                    for ei in range(NDT // 2):
                        et = eh * (NDT // 2) + ei
                        po = ps.tile([128, 512], f32, tag="w1", bufs=4,
                                     name=f"po{et}{sb}")
                        for ft in range(2):
                            nc.tensor.matmul(
                                po[:],
                                wo_sb[:, ft, et * 128:(et + 1) * 128],
                                y_t[ft][sb][:],
                                start=(ft == 0), stop=(ft == 1),
                            )
                        nc.vector.tensor_copy(o_sb[:, ei, :], po[:])
                    nc.sync.dma_start(
                        out.ap().rearrange("(et p) s -> p et s", p=128)[
                            :, eh * (NDT // 2):(eh + 1) * (NDT // 2),
                            sb * 512:(sb + 1) * 512],
                        o_sb[:],
                    )

            # ---- phase schedule ----
            qk_pass(wk_sb, bk_sb, xkt, kh_t, 0, "psk")
            qk_pass(wq_sb, bq_sb, xqt, qh_t, 0, "psq", gs=(0,))
            qk_pass(wk_sb, bk_sb, xkt, kh_t, 1, "psk")

            seq = [(0, 0), (0, 1), (0, 2), (0, 3), (1, 0), (1, 1), (1, 2), (1, 3)]
            pend = []   # queued PV work: (pr, ib, jc, at)
            pvmap = {}  # (pr, ib) -> psum pair

            def pop_one():
                ppr, pib, pjc, pat = pend.pop(0)
                pv_chunk(ppr, pvmap[(ppr, pib)], pat, pjc)
                if pjc == NST // 2 - 1:
                    finish_ib(ppr, pib, pvmap.pop((ppr, pib)),
                              with_outproj=(ppr == 1))

            for pr, ib in seq:
                pvmap[(pr, ib)] = [
                    ps.tile([128, 512], f32, tag="w1", bufs=4,
                            name=f"pv{pr}{ib}_{i}") for i in range(2)]
                first = (pr, ib) == (0, 0)
                for jc in range(NST // 2):
                    pend.append((pr, ib, jc, scores_chunk(pr, ib, jc)))
                    # (0,0) keeps full-ib lag (vh lands only after its v_pass);
                    # later ibs run self-interleaved at a steady PVLAG-chunk lag
                    if not first:
                        pops = 0
                        while len(pend) > PVLAG and pops < 2:
                            pop_one()
                            pops += 1
                if (pr, ib) == (0, 0):
                    v_pass(xvt)
                elif (pr, ib) == (0, 1):
                    qk_pass(wq_sb, bq_sb, xqt, qh_t, 0, "psq", gs=(1,))
                elif (pr, ib) == (0, 3):
                    qk_pass(wq_sb, bq_sb, xqt, qh_t, 1, "psq")

            while pend:
                pop_one()

    nc.compile()
    return nc


def _get_nc():
    if "nc" not in _compiled:
        _compiled["nc"] = _build()
    return _compiled["nc"]


def kernel(q, k, v, Wq, bq, Wk, bk, Wv, bv, Wo, bo):
    outp, _ = _run(q, k, v, Wq, bq, Wk, bk, Wv, bv, Wo, bo)
    return outp


def _run(q, k, v, Wq, bq, Wk, bk, Wv, bv, Wo, bo, **run_kwargs):
    from concourse.bass_utils import run_bass_kernel_spmd

    nc = _get_nc()

    q = np.asarray(q, np.float32)
    k = np.asarray(k, np.float32)
    v = np.asarray(v, np.float32)
    Wq = np.asarray(Wq, np.float32)
    Wk = np.asarray(Wk, np.float32)
    Wv = np.asarray(Wv, np.float32)
    Wo = np.asarray(Wo, np.float32)
    bq = np.asarray(bq, np.float32)
    bk = np.asarray(bk, np.float32)
    bv = np.asarray(bv, np.float32)
    bo = np.asarray(bo, np.float32)

    import ml_dtypes
    bf = ml_dtypes.bfloat16
    xqT = [np.ascontiguousarray(q[b].T).astype(bf) for b in range(B)]
    xkT = [np.ascontiguousarray(k[b].T).astype(bf) for b in range(B)]
    xvT = [np.ascontiguousarray(v[b].T).astype(bf) for b in range(B)]

    in_maps = []
    for c in range(NCORES):
        b, hg = divmod(c, HG)
        rows = slice(hg * F, (hg + 1) * F)
        in_maps.append({
            "xq": xqT[b], "xk": xkT[b], "xv": xvT[b],
            "wq": np.ascontiguousarray(Wq[rows].T).astype(bf),
            "wk": np.ascontiguousarray(Wk[rows].T).astype(bf),
            "wv": np.ascontiguousarray(Wv[rows].T).astype(bf),
            "wo": np.ascontiguousarray(Wo[:, rows].T).astype(bf),
            "bq": np.ascontiguousarray(bq[rows].reshape(2, 128).T),
            "bk": np.ascontiguousarray(bk[rows].reshape(2, 128).T),
            "bv": np.ascontiguousarray(bv[rows].reshape(1, F)),
        })

    res = run_bass_kernel_spmd(nc, in_maps, core_ids=list(range(NCORES)), **run_kwargs)

    outp = np.empty((B, S, D), np.float32)
    for b in range(B):
        acc = res.results[b * HG]["out"].astype(np.float32)
        for hg in range(1, HG):
            acc = acc + res.results[b * HG + hg]["out"].astype(np.float32)
        outp[b] = acc.T + bo[None, :]
    return outp, res


# revision 12
# speedup vs baseline: 1.0469x; 1.0469x over previous
"""Multi-head self-attention (B=2, S=2048, D=1024, H=16) on 8 TRN2 NeuronCores.

Sharding: data-parallel over batch (2) x tensor-parallel over head-groups (4).
Core c = b*4 + hg handles batch b, heads hg*4..hg*4+3 (4 heads, 256 features).

Per-core device program (SPMD, identical on all cores):
  - QKV projections for the core's 256 output features (column-parallel)
  - full S x S attention for its 4 heads (softmax without max-subtraction,
    denominators via an appended ones-column in the PV matmul)
  - partial output projection (row-parallel): out_partial^T [1024, 2048]
Host: shards/transposes inputs, sums the 4 partial outputs per batch
(the "all-reduce"), adds bo, and untransposes.

All matmuls run in bf16 on the PE; accumulation is fp32 in PSUM.

Schedule notes (from trace analysis):
  - DMA triggers cost ~0.6us each serialized on their queue; loads are split
    across the SP and ACT hardware-DGE queues with the kh critical path
    (wk, xk) first, and weight loads batched into one trigger each.
  - Output stores are batched per s-block (1 trigger instead of 8).
  - The softmax Exp on the ACT engine (1 elem/lane/cycle @1.2GHz) paces the
    attention phase; scores matmul pairs run concurrently via tile_position
    row-tiling, PV fills the gaps with a 3-chunk lag pipeline.
  - vh is padded to 128 columns (ones at col 64, zeros above) so the PV
    LDWEIGHTS takes the fast-weight-load path.
"""

import numpy as np

B, S, D = 2, 2048, 1024
H, DK = 16, 64
NCORES = 8
HG = 4          # head groups (tensor parallel)
HPG = 4         # heads per group
F = HPG * DK    # 256 local features per core
SCALE = 1.0 / np.sqrt(DK)

_compiled = {}


def _build():
    import concourse.bacc as bacc
    import concourse.tile as tile
    from concourse import mybir

    f32 = mybir.dt.float32
    bf16 = mybir.dt.bfloat16
    Exp = mybir.ActivationFunctionType.Exp
    mult = mybir.AluOpType.mult

    nc = bacc.Bacc("TRN2", target_bir_lowering=False, debug=False,
                   enable_asserts=False, num_devices=NCORES)

    xq = nc.dram_tensor("xq", (D, S), bf16, kind="ExternalInput")   # q[b].T
    xk = nc.dram_tensor("xk", (D, S), bf16, kind="ExternalInput")
    xv = nc.dram_tensor("xv", (D, S), bf16, kind="ExternalInput")
    wq = nc.dram_tensor("wq", (D, F), bf16, kind="ExternalInput")   # Wq[rows].T
    wk = nc.dram_tensor("wk", (D, F), bf16, kind="ExternalInput")
    wv = nc.dram_tensor("wv", (D, F), bf16, kind="ExternalInput")
    wo = nc.dram_tensor("wo", (F, D), bf16, kind="ExternalInput")   # Wo[:, cols].T
    bq = nc.dram_tensor("bq", (128, 2), f32, kind="ExternalInput")  # bias, f-tiled
    bk = nc.dram_tensor("bk", (128, 2), f32, kind="ExternalInput")
    bv = nc.dram_tensor("bv", (1, F), f32, kind="ExternalInput")
    out = nc.dram_tensor("out", (D, S), bf16, kind="ExternalOutput")  # partial^T

    NDT = D // 128   # 8 d-tiles
    NST = S // 128   # 16 s-tiles (j tiles)
    NSB = S // 512   # 4 s-blocks (i blocks)
    PVLAG = 3        # PV chunk-lag behind scores (chunks of 2 jt)

    with tile.TileContext(nc) as tc:
        import contextlib
        with contextlib.ExitStack() as ctx:
            consts = ctx.enter_context(tc.tile_pool(name="consts", bufs=1))
            big = ctx.enter_context(tc.tile_pool(name="big", bufs=24))
            atp = ctx.enter_context(tc.tile_pool(name="atp", bufs=9))
            acts = ctx.enter_context(tc.tile_pool(name="acts", bufs=1))
            ostage = ctx.enter_context(tc.tile_pool(name="ostage", bufs=2))
            small = ctx.enter_context(tc.tile_pool(name="small", bufs=1))
            ps = ctx.enter_context(tc.tile_pool(name="ps", bufs=1, space="PSUM"))

            # ---- constants (each weight loads with a single DMA trigger) ----
            wq_sb = consts.tile([128, NDT, F], bf16, tag="wq")
            wk_sb = consts.tile([128, NDT, F], bf16, tag="wk")
            wv_sb = consts.tile([128, NDT, F], bf16, tag="wv")
            wo_sb = consts.tile([128, 2, D], bf16, tag="wo")
            bq_sb = consts.tile([128, 2], f32, tag="bq")
            bk_sb = consts.tile([128, 2], f32, tag="bk")
            bv_sb = consts.tile([128, F], f32, tag="bv")

            # persistent activations
            # qh/kh: [f, s] transposed projections, per (ft, sb) tiles
            qh_t = [[acts.tile([128, 512], bf16, tag=f"qh{ft}{sb}", name=f"qh{ft}{sb}")
                     for sb in range(NSB)] for ft in range(2)]
            kh_t = [[acts.tile([128, 512], bf16, tag=f"kh{ft}{sb}", name=f"kh{ft}{sb}")
                     for sb in range(NSB)] for ft in range(2)]
            # vh: [s, h, c] padded to 128 cols: values at 0..63, ones at 64
            # (PV denominator trick), zeros at 65..127 so LDWEIGHTS uses the
            # 128-column fast path.
            vh_t = [acts.tile([128, HPG, 128], bf16, tag=f"vh{st}", name=f"vh{st}")
                    for st in range(NST)]
            for st in range(NST):
                nc.vector.memset(vh_t[st][:, :, DK:], 1.0)
            # y: normalized attention output, [f, s] per (ft, sb)
            y_t = [[acts.tile([128, 512], bf16, tag=f"y{ft}{sb}", name=f"y{ft}{sb}")
                    for sb in range(NSB)] for ft in range(2)]

            # ---- input DMAs: kh critical path (wk, xk) on the ACT hwdge
            # queue, everything else on SP, both issuing from t=0 ----
            def alloc_x(pfx):
                return [big.tile([128, S], bf16, tag="big", name=f"{pfx}{dt}")
                        for dt in range(NDT)]

            xkt = alloc_x("xk")
            xqt = alloc_x("xq")
            xvt = alloc_x("xv")

            def w_src(wdram):
                return wdram.ap().rearrange("(dt p) f -> p dt f", p=128)

            # kh and qh-first-half gate the first scores chunk: interleave
            # their input streams so both finish together
            nc.sync.dma_start(wk_sb[:], w_src(wk))
            nc.sync.dma_start(wq_sb[:], w_src(wq))
            nc.sync.dma_start(bk_sb[:], bk.ap()[:])
            nc.sync.dma_start(bq_sb[:], bq.ap()[:])
            for dt in range(NDT):
                nc.sync.dma_start(xkt[dt][:], xk.ap()[dt * 128:(dt + 1) * 128, :])
                nc.sync.dma_start(xqt[dt][:, 0:S // 2],
                                  xq.ap()[dt * 128:(dt + 1) * 128, 0:S // 2])
            nc.sync.dma_start(wv_sb[:], w_src(wv))
            nc.sync.dma_start(bv_sb[:], bv.ap().to_broadcast((128, F)))
            for dt in range(NDT):
                nc.sync.dma_start(xvt[dt][:], xv.ap()[dt * 128:(dt + 1) * 128, :])
            for dt in range(NDT):
                nc.sync.dma_start(xqt[dt][:, S // 2:S],
                                  xq.ap()[dt * 128:(dt + 1) * 128, S // 2:S])
            nc.sync.dma_start(wo_sb[:], wo.ap().rearrange("(ft p) d -> p ft d", p=128))

            # ---- projection pass helpers (per (ft, sb) single-bank accum) ----
            def qk_pass(w_sb, b_sb, xts, dst, ft, pfx, gs=(0, 1)):
                for g in gs:
                    accs = [ps.tile([128, 512], f32, tag="w1", bufs=4,
                                    name=f"{pfx}{ft}{g}{j}") for j in range(2)]
                    for dt in range(NDT):
                        for j in range(2):
                            sb = 2 * g + j
                            nc.tensor.matmul(
                                accs[j][:],
                                w_sb[:, dt, ft * 128:(ft + 1) * 128],
                                xts[dt][:, sb * 512:(sb + 1) * 512],
                                start=(dt == 0), stop=(dt == NDT - 1),
                            )
                    for j in range(2):
                        nc.vector.tensor_scalar_add(dst[ft][2 * g + j][:], accs[j][:],
                                                    b_sb[:, ft:ft + 1])

            def v_pass(xvt):
                for g in range(NST // 2):
                    accs = [ps.tile([128, 512], f32, tag="w1", bufs=4,
                                    name=f"vps{g}{j}") for j in range(2)]
                    for dt in range(NDT):
                        for j in range(2):
                            st = 2 * g + j
                            nc.tensor.matmul(
                                accs[j][:, 0:F],
                                xvt[dt][:, st * 128:(st + 1) * 128],
                                wv_sb[:, dt, :],
                                start=(dt == 0), stop=(dt == NDT - 1),
                            )
                    for j in range(2):
                        st = 2 * g + j
                        nc.vector.tensor_tensor(
                            vh_t[st][:, :, 0:DK],
                            accs[j][:, 0:F].rearrange("p (h c) -> p h c", h=HPG),
                            bv_sb[:].rearrange("p (h c) -> p h c", h=HPG),
                            mybir.AluOpType.add,
                        )

            # ---- attention pipeline: PV lags scores by PVLAG chunks so the
            # ACT-paced scores stream always has PE filler work ----
            def scores_chunk(pr, ib, jc):
                ft = pr
                at = atp.tile([128, 4, 512], bf16, tag="at", name=f"at{pr}{ib}{jc}")
                for jj in range(2):
                    jt = jc * 2 + jj
                    sc = ps.tile([128, 2, 512], f32, tag="w2", bufs=2, name="sc")
                    for hh in range(2):
                        base = hh * 64
                        nc.tensor.matmul(
                            sc[:, hh, :],
                            kh_t[ft][jt // 4][base:base + 64,
                                              (jt % 4) * 128:(jt % 4 + 1) * 128],
                            qh_t[ft][ib][base:base + 64, :],
                            start=True, stop=True,
                            tile_position=(base, 0),
                        )
                    nc.scalar.activation(
                        at[:, jj * 2:jj * 2 + 2, :],
                        sc[:, :, :],
                        Exp, scale=float(SCALE),
                    )
                return at

            def pv_chunk(pr, pv_ps, at, jc):
                for hh in range(2):
                    h = 2 * pr + hh
                    for jj in range(2):
                        jt = 2 * jc + jj
                        nc.tensor.matmul(
                            pv_ps[hh][:],
                            vh_t[jt][:, h, :],
                            at[:, 2 * jj + hh, :],
                            start=(jt == 0), stop=(jt == NST - 1),
                        )

            def finish_ib(pr, ib, pv_ps, with_outproj=False):
                ft = pr
                for hh in range(2):
                    den = small.tile([1, 512], f32, tag="den")
                    nc.vector.tensor_copy(den[:], pv_ps[hh][DK:DK + 1, :])
                    rec = small.tile([1, 512], f32, tag="rec")
                    nc.vector.reciprocal_approx_fast(rec[:], den[:])
                    rb = small.tile([64, 512], f32, tag="rb")
                    nc.gpsimd.partition_broadcast(rb[:], rec[:])
                    nc.vector.tensor_tensor(
                        y_t[ft][ib][hh * 64:hh * 64 + 64, :],
                        pv_ps[hh][0:DK, :],
                        rb[:],
                        mult,
                    )
                if with_outproj:
                    outproj_sb(ib)

            # ---- output projection for one s-block (single batched store) ----
            def outproj_sb(sb):
                for eh in range(2):
                    o_sb = ostage.tile([128, NDT // 2, 512], bf16, tag="ost",
                                       name=f"os{sb}{eh}")
                    for ei in range(NDT // 2):
                        et = eh * (NDT // 2) + ei
                        po = ps.tile([128, 512], f32, tag="w1", bufs=4,
                                     name=f"po{et}{sb}")
                        for ft in range(2):
                            nc.tensor.matmul(
                                po[:],
                                wo_sb[:, ft, et * 128:(et + 1) * 128],
                                y_t[ft][sb][:],
                                start=(ft == 0), stop=(ft == 1),
                            )
                        nc.vector.tensor_copy(o_sb[:, ei, :], po[:])
                    nc.sync.dma_start(
                        out.ap().rearrange("(et p) s -> p et s", p=128)[
                            :, eh * (NDT // 2):(eh + 1) * (NDT // 2),
                            sb * 512:(sb + 1) * 512],
                        o_sb[:],
                    )

            # ---- phase schedule ----
            qk_pass(wk_sb, bk_sb, xkt, kh_t, 0, "psk")
            qk_pass(wq_sb, bq_sb, xqt, qh_t, 0, "psq", gs=(0,))
            qk_pass(wk_sb, bk_sb, xkt, kh_t, 1, "psk")

            seq = [(0, 0), (0, 1), (0, 2), (0, 3), (1, 0), (1, 1), (1, 2), (1, 3)]
            prev = None  # (pr, ib, pv_ps, at_list)
            lpv = None
            for pr, ib in seq:
                last = (pr, ib) == seq[-1]
                at_list = []
                ppv = None
                for jc in range(NST // 2):
                    at_list.append(scores_chunk(pr, ib, jc))
                    if prev is not None:
                        if jc == 0:
                            ppv = [ps.tile([128, 512], f32, tag="w1", bufs=4,
                                           name=f"pv{prev[0]}{prev[1]}_{i}")
                                   for i in range(2)]
                        pv_chunk(prev[0], ppv, prev[3][jc], jc)
                    if last and jc >= PVLAG:
                        if lpv is None:
                            lpv = [ps.tile([128, 512], f32, tag="w1", bufs=4,
                                           name=f"pvlast{i}") for i in range(2)]
                        pv_chunk(pr, lpv, at_list[jc - PVLAG], jc - PVLAG)
                if last:
                    # drain this ib's remaining PV, release both PV psum
                    # pairs, then run the two tail outprojs
                    for jc in range(NST // 2 - PVLAG, NST // 2):
                        pv_chunk(pr, lpv, at_list[jc], jc)
                    finish_ib(prev[0], prev[1], ppv, with_outproj=False)
                    finish_ib(pr, ib, lpv, with_outproj=False)
                    outproj_sb(prev[1])
                    outproj_sb(ib)
                elif prev is not None:
                    finish_ib(prev[0], prev[1], ppv, with_outproj=(prev[0] == 1))
                if (pr, ib) == (0, 0):
                    v_pass(xvt)
                elif (pr, ib) == (0, 1):
                    qk_pass(wq_sb, bq_sb, xqt, qh_t, 0, "psq", gs=(1,))
                elif (pr, ib) == (0, 3):
                    qk_pass(wq_sb, bq_sb, xqt, qh_t, 1, "psq")
                prev = (pr, ib, None, at_list)

    nc.compile()
    return nc


def _get_nc():
    if "nc" not in _compiled:
        _compiled["nc"] = _build()
    return _compiled["nc"]


def kernel(q, k, v, Wq, bq, Wk, bk, Wv, bv, Wo, bo):
    outp, _ = _run(q, k, v, Wq, bq, Wk, bk, Wv, bv, Wo, bo)
    return outp


def _run(q, k, v, Wq, bq, Wk, bk, Wv, bv, Wo, bo, **run_kwargs):
    from concourse.bass_utils import run_bass_kernel_spmd

    nc = _get_nc()

    q = np.asarray(q, np.float32)
    k = np.asarray(k, np.float32)
    v = np.asarray(v, np.float32)
    Wq = np.asarray(Wq, np.float32)
    Wk = np.asarray(Wk, np.float32)
    Wv = np.asarray(Wv, np.float32)
    Wo = np.asarray(Wo, np.float32)
    bq = np.asarray(bq, np.float32)
    bk = np.asarray(bk, np.float32)
    bv = np.asarray(bv, np.float32)
    bo = np.asarray(bo, np.float32)

    import ml_dtypes
    bf = ml_dtypes.bfloat16
    xqT = [np.ascontiguousarray(q[b].T).astype(bf) for b in range(B)]
    xkT = [np.ascontiguousarray(k[b].T).astype(bf) for b in range(B)]
    xvT = [np.ascontiguousarray(v[b].T).astype(bf) for b in range(B)]

    in_maps = []
    for c in range(NCORES):
        b, hg = divmod(c, HG)
        rows = slice(hg * F, (hg + 1) * F)
        in_maps.append({
            "xq": xqT[b], "xk": xkT[b], "xv": xvT[b],
            "wq": np.ascontiguousarray(Wq[rows].T).astype(bf),
            "wk": np.ascontiguousarray(Wk[rows].T).astype(bf),
            "wv": np.ascontiguousarray(Wv[rows].T).astype(bf),
            "wo": np.ascontiguousarray(Wo[:, rows].T).astype(bf),
            "bq": np.ascontiguousarray(bq[rows].reshape(2, 128).T),
            "bk": np.ascontiguousarray(bk[rows].reshape(2, 128).T),
            "bv": np.ascontiguousarray(bv[rows].reshape(1, F)),
        })

    res = run_bass_kernel_spmd(nc, in_maps, core_ids=list(range(NCORES)), **run_kwargs)

    outp = np.empty((B, S, D), np.float32)
    for b in range(B):
        acc = res.results[b * HG]["out"].astype(np.float32)
        for hg in range(1, HG):
            acc = acc + res.results[b * HG + hg]["out"].astype(np.float32)
        outp[b] = acc.T + bo[None, :]
    return outp, res
